# revision 9
# baseline (speedup 1.0000x reference)
"""AxialAttention TRN2 Bass kernel.

Shapes (hardcoded): x [B=4,T=16,C=256,H=64,W=64] fp32.
N = B*T*H = 4096 lines of [L=64, C=256]; heads=8, d=32.
Sharding: 64 (b,t) blocks -> 8 per core across 8 cores (data parallel).

Per-core dataflow — software-pipelined at group granularity (a group is
8 lines = 512 positions); projections for group gi run 2 groups ahead of
attention for group gi-2, interleaved in one loop so PE stays dense and
ACT/DVE evacuation load is spread evenly:

  qkT  = w_qk^T @ xT      fp32r MMs N=512, q pre-scaled by 1/sqrt(d);
                          b_q folded into the psum->sbuf evacuation.
  v    = xT^T @ w_v       row-major v; v_sw = partition-swapped copy (DMA);
                          v_bd = per-line block-diagonal tiles built by 4
                          strided GPSIMD copies into a pre-zeroed ring of
                          raw sbuf tensors ([d_b | d_b+4] columns, zeros
                          in the cross blocks).
  per group g, half hf (banks b = 2hf+bw, heads {2hf+bw, 2hf+4+bw}):
    scoresT[k,q] = kT_h^T @ qT_h      bf16, tile-position packed, one
                                      [128,512] psum bank per (hf,bw)
    praw  = exp(scoresT)              ACT, per bank
    probs = praw * exp(bias)          DVE bf16 (replaces psum bias seeds)
    Zbc   = Bz^T @ probs              2 MMs -> broadcast Z [128,512]
                                      (0/1 Bz matrices; no compact-Z /
                                      broadcast stage needed)
    rzbc  = 1/Zbc                     DVE reciprocal_approx_fast
    oT    = v_bd^T @ probs            16 MMs K=128 M=64 N=64, rows
                                      [h, h+4, h+1, h+5] per chunk
    oT_n  = oT * rzbc                 DVE
    proj += wp[hf]^T @ oT_n           fp32r, accumulated over halves
  out = proj + (b_v @ w_proj + b_proj)   evacuated one group deferred
                                         (ACT mc0 / DVE mc1), then DMA.

PSUM (16KB/partition, exactly full): 4x [128,512] "sc" (scores + q/k
projection chunks), 1x [128,1024] "o" (attnv out pair), 1x [128,512] "z"
(Zbc + k-proj mc2 + out-proj mc1), 1x [128,512] "proj" (k-proj mc3 +
out-proj mc0).
"""

import numpy as np

B, T, C, H, W = 4, 16, 256, 64, 64
HEADS, D = 8, 32
NBT = B * T
NCORES = 8
BT_PER_CORE = NBT // NCORES  # 8
HW = H * W                   # 4096
L = W
GRP = 8
NGRP = H // GRP              # 8
GQ = GRP * L                 # 512

ST_COLS = 4356
VBD_BUFS = 8


def _build_bass():
    import concourse.bacc as bacc
    import concourse.mybir as mybir
    from concourse.tile import TileContext

    f32 = mybir.dt.float32
    f32r = mybir.dt.float32r
    bf16 = mybir.dt.bfloat16
    fp16 = mybir.dt.float16
    AF = mybir.ActivationFunctionType

    nc = bacc.Bacc("TRN2", target_bir_lowering=False, debug=False,
                   num_devices=NCORES)

    x_d = nc.dram_tensor("x", [BT_PER_CORE, C, HW], f32r, kind="ExternalInput").ap()
    st_d = nc.dram_tensor("statics", [128, ST_COLS], f32r, kind="ExternalInput").ap()
    out_d = nc.dram_tensor("out", [BT_PER_CORE, C, HW], f32, kind="ExternalOutput").ap()

    with TileContext(nc) as tc:
        with (
            tc.tile_pool(name="static", bufs=1) as stat,
            tc.tile_pool(name="xt", bufs=5) as pxt,
            tc.tile_pool(name="qk", bufs=28) as pqk,
            tc.tile_pool(name="vg", bufs=6) as pvg,
            tc.tile_pool(name="probs", bufs=8) as ppr,
            tc.tile_pool(name="rz", bufs=3) as prz,
            tc.tile_pool(name="osb", bufs=4) as po,
            tc.tile_pool(name="outsb", bufs=4) as pout,
            tc.tile_pool(name="psS", bufs=4, space="PSUM") as psS,
            tc.tile_pool(name="psZ", bufs=1, space="PSUM") as psZ,
            tc.tile_pool(name="psO", bufs=2, space="PSUM") as psO,
            tc.tile_pool(name="psP", bufs=1, space="PSUM") as psP,
        ):
            # ---- statics ----
            st = stat.tile([128, ST_COLS], f32r, tag="st", name="statics_sb")
            nc.sync.dma_start(out=st[:, 0:1024], in_=st_d[:, 0:1024])
            nc.sync.dma_start(out=st[:, 1024:2048], in_=st_d[:, 1024:2048])
            nc.sync.dma_start(out=st[:, 4352:4356], in_=st_d[:, 4352:4356])
            wqk = [st[:, 512 * i:512 * (i + 1)] for i in range(2)]
            wv = [st[:, 1024 + 256 * i:1024 + 256 * (i + 1)] for i in range(2)]
            wp = [[st[:, 1536 + 256 * i + 128 * j:1536 + 256 * i + 128 * (j + 1)]
                   for j in range(2)] for i in range(2)]
            expb_st = st[:, 2048:4096]
            bz_st = st[:, 4096:4352]
            bq = st[:, 4352:4354].bitcast(f32)
            bp = st[:, 4354:4356].bitcast(f32)
            expb_bf = stat.tile([128, 2048], bf16, tag="ebbf", name="expb_bf")
            bz_bf = stat.tile([128, 256], bf16, tag="bzbf", name="bz_bf")

            # v_bd ring: raw sbuf tensors, zero blocks memset once and
            # never rewritten (Tile tracks WAR hazards by address).
            vbd_ring = [nc.alloc_sbuf_tensor(f"vbd{i}", [128, 2048], bf16).ap()
                        for i in range(VBD_BUFS)]
            for t in vbd_ring:
                nc.gpsimd.memset(t, 0.0)

            deferred = []
            for bt in range(BT_PER_CORE):
                # ---- load xT in [128, 2048] slices: xt[kc][xh] ----
                xt = [[pxt.tile([128, 2048], f32r, tag="xt", name="xt")
                       for _ in range(2)] for _ in range(2)]
                for xh in range(2):
                    for kc in range(2):
                        if bt == 0 and xh == 0:
                            nc.sync.dma_start(
                                out=xt[kc][xh][:, 0:512],
                                in_=x_d[bt, 128 * kc:128 * (kc + 1), 0:512])
                            nc.sync.dma_start(
                                out=xt[kc][xh][:, 512:2048],
                                in_=x_d[bt, 128 * kc:128 * (kc + 1),
                                        512:2048])
                        else:
                            nc.sync.dma_start(
                                out=xt[kc][xh],
                                in_=x_d[bt, 128 * kc:128 * (kc + 1),
                                        2048 * xh:2048 * (xh + 1)])

                if bt == 0:
                    # expb/bz arrive after block-0 inputs (needed ~12us in)
                    nc.sync.dma_start(out=st[:, 2048:4352],
                                      in_=st_d[:, 2048:4352])
                    with nc.allow_low_precision(reason="exact 0/1 + bias"):
                        nc.vector.tensor_copy(expb_bf, expb_st)
                        nc.vector.tensor_copy(bz_bf, bz_st)

                def xt_cols(kc, c0, w_):
                    xh = c0 // 2048
                    o = c0 - 2048 * xh
                    return xt[kc][xh][:, o:o + w_]

                # ---- interleaved qk + v projections ----
                # qk: per (mc, nn) -> qkt[(mc,nn)] bf16 [128,512]
                # v:  per group -> v_g -> v_sw (DMA) -> v_bd (gpsimd copies)
                qkt = {}
                v_bd = []

                def qk_tile(g, mc):
                    # psum: mc0/1 from the sc pool; mc2 -> psZ, mc3 -> psP
                    # (those pools idle at this point of the group cycle).
                    if mc < 2:
                        ps = psO.tile([128, 512], f32, tag="o", name="psqk")
                    elif mc == 2:
                        ps = psZ.tile([128, 512], f32, tag="z", name="psqk")
                    else:
                        ps = psP.tile([128, 512], f32, tag="proj", name="psqk")
                    for kc in range(2):
                        nc.tensor.matmul(
                            ps,
                            wqk[kc][:, 128 * mc:128 * (mc + 1)],
                            xt_cols(kc, 512 * g, 512),
                            start=(kc == 0), stop=(kc == 1))
                    dst = pqk.tile([128, 512], bf16, tag="qkT", name="qkT")
                    qkt[(mc, g)] = dst
                    if mc < 2:   # q: fold b_q; mc alternates engine
                        if mc == 0:
                            nc.scalar.activation(
                                dst, ps, AF.Identity,
                                bias=bq[:, mc:mc + 1], scale=1.0)
                        else:
                            with nc.allow_low_precision(reason="bias add"):
                                nc.vector.tensor_scalar_add(
                                    dst, ps, bq[:, mc:mc + 1])
                    else:        # k: plain copy; mc alternates engine
                        if mc == 2:
                            nc.scalar.copy(dst, ps)
                        else:
                            nc.vector.tensor_copy(dst, ps)

                def v_tile(g):
                    ph = (psS.tile([128, 512], f32, tag="sc", name="psv"),
                          psS.tile([128, 512], f32, tag="sc", name="psv"))
                    for pl in range(4):
                        pc = 4 * g + pl
                        for kc in range(2):
                            nc.tensor.matmul(
                                ph[pl // 2][:, 256 * (pl % 2):256 * (pl % 2 + 1)],
                                xt_cols(kc, 128 * pc, 128),
                                wv[kc], start=(kc == 0), stop=(kc == 1))
                    vg = pvg.tile([128, 1024], bf16, tag="v", name="v")
                    nc.scalar.copy(vg[:, 0:512], ph[0])
                    nc.scalar.copy(vg[:, 512:1024], ph[1])
                    vs = pvg.tile([128, 1024], bf16, tag="vsw", name="vsw")
                    nc.sync.dma_start(out=vs[0:64, :], in_=vg[64:128, :])
                    nc.sync.dma_start(out=vs[64:128, :], in_=vg[0:64, :])
                    vb = vbd_ring[(bt * NGRP + g) % VBD_BUFS]
                    # 4 partition-aligned strided copies on idle GPSIMD:
                    # (tb, p2): dst row-half tb, line parity p2; source vg
                    # when tb==p2 else the swapped copy vs.
                    for tb in range(2):
                        dst = vb[64 * tb:64 * (tb + 1), :].rearrange(
                            "p (pc l b j) -> p pc l b j", pc=4, l=2, b=4, j=64)
                        for p2 in range(2):
                            srcT = vg if tb == p2 else vs
                            src = srcT[64 * tb:64 * (tb + 1), :].rearrange(
                                "p (c hb j) -> p c hb j", c=4, hb=8, j=32)
                            with nc.allow_low_precision(reason="bf16 copy"):
                                nc.gpsimd.tensor_copy(
                                    dst[:, :, p2, :, 32 * tb:32 * (tb + 1)],
                                    src[:, :, 4 * tb:4 * (tb + 1), :])
                    v_bd.append(vb)

                # ---- per-group: projections then attention ----
                def flush_out(dfr):
                    dbt, dg, pps = dfr
                    for mc in range(2):
                        osb = pout.tile([128, 512], f32, tag="out", name="outsb")
                        if mc == 0:
                            nc.scalar.activation(
                                osb, pps[mc], AF.Identity,
                                bias=bp[:, mc:mc + 1], scale=1.0)
                        else:
                            with nc.allow_low_precision(reason="f32 add"):
                                nc.vector.tensor_scalar_add(
                                    osb, pps[mc], bp[:, mc:mc + 1])
                        nc.sync.dma_start(
                            out=out_d[dbt, 128 * mc:128 * (mc + 1),
                                      GQ * dg:GQ * (dg + 1)],
                            in_=osb)

                for gi in range(NGRP + 2):
                    g = gi - 2
                    if g < 0:
                        for mc in range(4):
                            qk_tile(gi, mc)
                        v_tile(gi)
                        continue
                    probs = []
                    for hf in range(2):
                        prr = ppr.tile([128, 1024], bf16, tag="praw", name="praw")
                        pr = ppr.tile([128, 1024], bf16, tag="probs", name="probs")
                        probs.append(pr)
                        for bw in range(2):
                            sp = psS.tile([128, 512], f32, tag="sc", name="psatt")
                            for li in range(GRP):
                                for h in (2 * hf + bw, 2 * hf + 4 + bw):
                                    hc, hr = h // 4, h % 4
                                    kt = qkt[(2 + hc, g)][32 * hr:32 * (hr + 1),
                                                          64 * li:64 * (li + 1)]
                                    qt = qkt[(hc, g)][32 * hr:32 * (hr + 1),
                                                      64 * li:64 * (li + 1)]
                                    nc.tensor.matmul(
                                        sp[64 * hc:64 * (hc + 1),
                                           64 * li:64 * (li + 1)],
                                        kt, qt, start=True, stop=True,
                                        tile_position=(32 * hr, 64 * hc))
                            nc.scalar.activation(
                                prr[:, 512 * bw:512 * (bw + 1)],
                                sp, AF.Exp, scale=1.0)
                            with nc.allow_low_precision(reason="bf16 probs"):
                                nc.vector.tensor_mul(
                                    pr[:, 512 * bw:512 * (bw + 1)],
                                    prr[:, 512 * bw:512 * (bw + 1)],
                                    expb_bf[:, 1024 * hf + 512 * bw:
                                            1024 * hf + 512 * (bw + 1)])
                            if hf == 0 and bw == 0 and deferred:
                                flush_out(deferred.pop(0))

                    if gi < NGRP:
                        for mc in range(4):
                            qk_tile(gi, mc)
                        v_tile(gi)

                    oT = []
                    for hf in range(2):
                        ops = psO.tile([128, 512], f32, tag="o", name="pso")
                        # attnv first: only needs probs
                        for bw in range(2):
                            for li in range(GRP):
                                b = 2 * hf + bw
                                vt = v_bd[g][:, 256 * li + 64 * b:
                                             256 * li + 64 * (b + 1)]
                                pt = probs[hf][:, 512 * bw + 64 * li:
                                               512 * bw + 64 * (li + 1)]
                                nc.tensor.matmul(
                                    ops[64 * bw:64 * (bw + 1),
                                        64 * li:64 * (li + 1)],
                                    vt, pt, start=True, stop=True,
                                    tile_position=(0, 64 * bw))
                        zps = psZ.tile([128, 512], f32, tag="z", name="psz")
                        # (psZ also hosts qk-mc2 and proj-mc1 tiles)
                        for bw in range(2):
                            nc.tensor.matmul(
                                zps, bz_bf[:, 128 * bw:128 * (bw + 1)],
                                probs[hf][:, 512 * bw:512 * (bw + 1)],
                                start=(bw == 0), stop=(bw == 1))
                        rz = prz.tile([128, 512], f32, tag="rz", name="rz")
                        nc.vector.reciprocal_approx_fast(out=rz, in_=zps)
                        o = po.tile([128, 512], f32r, tag="oT", name="oT")
                        with nc.allow_low_precision(reason="f32r bits are f32"):
                            nc.vector.tensor_mul(o, ops, rz)
                        oT.append(o)

                    pps = []
                    for mc in range(2):
                        if mc == 0:
                            pp = psP.tile([128, 512], f32, tag="proj", name="pspr")
                        else:
                            pp = psZ.tile([128, 512], f32, tag="z", name="pspr2")
                        for hf in range(2):
                            nc.tensor.matmul(pp, wp[hf][mc], oT[hf],
                                             start=(hf == 0), stop=(hf == 1))
                        pps.append(pp)
                    deferred.append((bt, g, pps))
            for dfr in deferred:
                flush_out(dfr)
            deferred = []
    nc.compile()
    return nc


def _host_inputs(x, relative_bias, w_qkv, b_qkv, w_proj, b_proj):
    scale = D ** -0.5
    wq = w_qkv[:, :C] * scale
    wk = w_qkv[:, C:2 * C]
    wvm = w_qkv[:, 2 * C:]
    bqv = b_qkv[:C] * scale
    bv = b_qkv[2 * C:]
    wqk_full = np.concatenate([wq, wk], axis=1)
    perm = []
    for c_ in range(2):
        for h in (2 * c_, 2 * c_ + 4, 2 * c_ + 1, 2 * c_ + 5):
            perm.extend(range(32 * h, 32 * (h + 1)))
    wp_perm = w_proj[perm, :]
    # expb [128, 2048]: per (hf, bw): bank b = 2hf+bw, head h = b+4hh:
    # expb[64hh+k, 1024hf+512bw+64li+q] = exp(bias[h][q, k]), replicated
    # over the 8 lines of a group.
    expb = np.zeros((128, 2048), np.float32)
    for hf in range(2):
        for bw in range(2):
            b = 2 * hf + bw
            for hh in range(2):
                blk = np.exp(relative_bias[b + 4 * hh].T)      # [k, q]
                expb[64 * hh:64 * (hh + 1),
                     1024 * hf + 512 * bw:1024 * hf + 512 * (bw + 1)] = \
                    np.tile(blk, (1, GRP))
    bz = np.zeros((128, 256), np.float32)
    bz[0:64, 0:32] = 1.0
    bz[64:128, 32:64] = 1.0
    bz[0:64, 128 + 64:128 + 96] = 1.0
    bz[64:128, 128 + 96:128 + 128] = 1.0
    bq = np.stack([bqv[:128], bqv[128:]], axis=1).astype(np.float32)
    bpv = bv @ w_proj + b_proj
    bp = np.stack([bpv[:128], bpv[128:]], axis=1).astype(np.float32)
    st = np.zeros((128, ST_COLS), np.float32)
    st[:, 0:512] = wqk_full[:128]
    st[:, 512:1024] = wqk_full[128:]
    st[:, 1024:1280] = wvm[:128]
    st[:, 1280:1536] = wvm[128:]
    for kc in range(2):
        for mc in range(2):
            st[:, 1536 + 256 * kc + 128 * mc:1536 + 256 * kc + 128 * (mc + 1)] = \
                wp_perm[128 * kc:128 * (kc + 1), 128 * mc:128 * (mc + 1)]
    st[:, 2048:4096] = expb
    st[:, 4096:4352] = bz
    st[:, 4352:4354] = bq
    st[:, 4354:4356] = bp
    return dict(statics=st)


LAST_RESULTS = None


def kernel(x, relative_bias, w_qkv, b_qkv, w_proj, b_proj):
    import os
    import sys
    if '/opt/trn_rl_repo' not in sys.path:
        sys.path.insert(0, '/opt/trn_rl_repo')
    from concourse.bass_utils import run_bass_kernel_spmd

    x = np.asarray(x, np.float32)
    const = _host_inputs(x,
                         np.asarray(relative_bias, np.float32),
                         np.asarray(w_qkv, np.float32),
                         np.asarray(b_qkv, np.float32),
                         np.asarray(w_proj, np.float32),
                         np.asarray(b_proj, np.float32))
    xr = np.ascontiguousarray(x.reshape(NBT, C, HW))
    nc = _build_bass()
    in_maps = []
    for c in range(NCORES):
        m = dict(const)
        m["x"] = np.ascontiguousarray(xr[c * BT_PER_CORE:(c + 1) * BT_PER_CORE])
        in_maps.append(m)
    res = run_bass_kernel_spmd(nc, in_maps, list(range(NCORES)),
                               tmpdir=os.environ.get("BASS_TMPDIR"))
    global LAST_RESULTS
    LAST_RESULTS = res
    outs = res.results
    out = np.concatenate([o["out"].reshape(BT_PER_CORE, C, HW) for o in outs],
                         axis=0)
    return out.reshape(B, T, C, H, W).astype(np.float32)
